# revision 1
# baseline (speedup 1.0000x reference)
"""Trainium2 Bass kernel for nn_CascadedVMambaBlock.

Sharding: 8 cores; core c = (b, nh) with b = c//4, nh = c%4.
Each core processes sample b with state-dim slice n in [4nh, 4nh+4)
for ALL 4 scan directions k; a per-stage AllReduce over each 4-core
b-group combines the n-partials of y.

Everything on-device is channels-first: (channels on partitions,
L = 48*48 = 2304 pixels on the free dim).

Per (k, gl) scan-lane layout (gl in {0,1}): partition p = half*64 + d
covers state n = 4*nh + 2*gl + half, channel d = p % 64.
The selective-scan recurrence runs as a single hardware instruction
per (128, chunk): tensor_tensor_scan computes state = dA*state + bB
along the free dim. k in {2,3} scan backwards via negative-step APs.

LayerNorms run channels-first using PE for the channel reductions
(masked-ones lhsT matmuls) and rstd = Exp(-0.5*Ln(var+eps)) on ACT
(no rsqrt table needed). ln1 gamma/beta are folded into in_proj on
the host; out_norm gamma into out_proj.
"""
import numpy as np

# problem constants (hardcoded; kernel.py must be self-contained)
HEAD, C_IN, C_H = 4, 128, 32
D, N, K, DT_RANK = 64, 16, 4, 2
B, H, W = 2, 48, 48
L = H * W            # 2304
CS = 512             # chunk size along L (LN phases)
CHUNKS = [(i * CS, min(CS, L - i * CS)) for i in range((L + CS - 1) // CS)]
NCH = len(CHUNKS)    # 5
SCS = 1024           # scan-core chunk size (bf16 moving operands)
SCHUNKS = [(i * SCS, min(SCS, L - i * SCS)) for i in range((L + SCS - 1) // SCS)]
EPS = 1e-5
NCORES = 8

_cache = {}


def _build(vs, cvm):
    import concourse.bass as bass
    import concourse.bacc as bacc
    import concourse.tile as tile
    import concourse.mybir as mybir
    from contextlib import ExitStack

    f32 = mybir.dt.float32
    f32r = mybir.dt.float32r
    bf16 = mybir.dt.bfloat16
    AF = mybir.ActivationFunctionType
    OP = mybir.AluOpType

    # Force the ACT-table chooser to a single exp/ln table (+ the silu
    # table): without this, Ln resolves to 'natural_log' and Exp to
    # 'natural_log_exp_and_others', and alternating Ln/Exp ops thrash
    # ACT_TABLE_LOADs (~1.3us each, hundreds of loads).
    import concourse.hw_specs as hw_specs
    _orig_gat = hw_specs.get_activation_tables
    _KEEP = {"natural_log_exp_and_others", "silu_and_others"}

    def _patched_gat(arch):
        t = _orig_gat(arch)
        return {k: (v if k in _KEEP else set()) for k, v in t.items()}

    bacc.get_activation_tables = _patched_gat

    nc = bacc.Bacc("TRN2", target_bir_lowering=False, debug=False,
                   enable_asserts=True, num_devices=NCORES)

    # inputs (per-core host-prepped tensors)
    def din(name, shape, dtype=f32):
        return nc.dram_tensor(name, shape, dtype, kind="ExternalInput").ap()

    x_shuf_d = din("x_shuf", (C_IN, L))                # shuffled input (final res)
    lhsT_ip_d = din("lhsT_ip", (C_H, C_IN), f32r)      # in_proj lhsT (32,128)
    lhsT_nw_d = din("lhsT_nw", (1, C_IN))              # -colsum(W_ip) row
    bias_xx_d = din("bias_xx", (D, 1))                 # in_proj bias rows 0:64
    bias_z_d = din("bias_z", (D, 1))                   # rows 64:128 (at base 0)
    w9_d = din("w9", (D, 9))                           # conv taps per channel
    convb_d = din("convb", (D, 1))
    lhsT_st1_d = din("lhsT_st1", (C_H, 1), f32r)       # ones/32
    lhsT_bc_d = din("lhsT_bc", (1, C_IN))              # ones row (broadcasts)
    lhsT_M2_d = din("lhsT_M2", (D, K, C_IN), bf16)     # dt fused proj, doubled
    dtb2_d = din("dtb2", (C_IN, K))                    # dt bias doubled
    A2_d = din("A2", (C_IN, K, 2))                     # A scale per (k, gl)
    lhsT_B_d = din("lhsT_B", (D, K, 2, C_IN), bf16)    # B_exp weights
    lhsT_C_d = din("lhsT_C", (D, K, 2, C_IN), bf16)    # C_exp weights
    lhsT_ys_d = din("lhsT_ys", (C_IN, D), bf16)        # half-sum (p%64==d)
    Ds_q_d = din("Ds_q", (D, 1))                       # sum_k Ds / 4
    lhsT_st64_d = din("lhsT_st64", (C_IN, 2), f32r)    # out_norm stats masks
    lhsT_op_d = din("lhsT_op", (D, C_H), f32r)         # out_proj lhsT
    lhsT_st128_d = din("lhsT_st128", (C_IN, 2), f32r)  # final stats masks
    gamma_d = din("gamma", (C_IN, 1))
    beta_d = din("beta", (C_IN, 1))

    out_d = nc.dram_tensor("out_cf", (C_IN, L), f32, kind="ExternalOutput").ap()

    with tile.TileContext(nc) as tc, ExitStack() as ctx:
        # pools
        w_pool = ctx.enter_context(tc.tile_pool(name="weights", bufs=1))
        big = ctx.enter_context(tc.tile_pool(name="big", bufs=1))
        stg = ctx.enter_context(tc.tile_pool(name="stg", bufs=1))
        sml = ctx.enter_context(tc.tile_pool(name="sml", bufs=2))
        scn = ctx.enter_context(tc.tile_pool(name="scn", bufs=2))
        hpool = ctx.enter_context(tc.tile_pool(name="hpool", bufs=5))
        ps = ctx.enter_context(tc.tile_pool(name="ps", bufs=1, space="PSUM"))
        dram = ctx.enter_context(tc.tile_pool(name="dram", bufs=2, space="DRAM"))

        # load weights/inputs
        def wload(ap_d, shape, dtype=f32):
            t = w_pool.tile(list(shape), dtype, name=ap_d.tensor.name + "_sb")
            src = ap_d if ap_d.dtype == dtype else ap_d.bitcast(dtype)
            nc.sync.dma_start(t[:], src)
            return t

        x_shuf = wload(x_shuf_d, (C_IN, L), f32r)
        lhsT_ip = wload(lhsT_ip_d, (C_H, C_IN), f32r)
        lhsT_nw = wload(lhsT_nw_d, (1, C_IN))
        bias_xx = wload(bias_xx_d, (D, 1))
        bias_z = wload(bias_z_d, (D, 1))
        w9 = wload(w9_d, (D, 9))
        convb = wload(convb_d, (D, 1))
        lhsT_st1 = wload(lhsT_st1_d, (C_H, 1), f32r)
        lhsT_bc = wload(lhsT_bc_d, (1, C_IN))
        lhsT_M2 = wload(lhsT_M2_d, (D, K, C_IN), bf16)
        dtb2 = wload(dtb2_d, (C_IN, K))
        A2 = wload(A2_d, (C_IN, K, 2))
        lhsT_B = wload(lhsT_B_d, (D, K, 2, C_IN), bf16)
        lhsT_C = wload(lhsT_C_d, (D, K, 2, C_IN), bf16)
        lhsT_ys = wload(lhsT_ys_d, (C_IN, D), bf16)
        Ds_q = wload(Ds_q_d, (D, 1))
        lhsT_st64 = wload(lhsT_st64_d, (C_IN, 2), f32r)
        lhsT_op = wload(lhsT_op_d, (D, C_H), f32r)
        lhsT_st128 = wload(lhsT_st128_d, (C_IN, 2), f32r)
        gamma = wload(gamma_d, (C_IN, 1))
        beta = wload(beta_d, (C_IN, 1))

        # persistent big tensors
        xx_pad = big.tile([D, 50 * 50], f32)       # zero-padded conv input
        nc.vector.memset(xx_pad[:], 0.0)
        xx_int = xx_pad[:].rearrange("c (h w) -> c h w", h=50, w=50)[:, 1:49, 1:49]
        outs_cat = big.tile([C_IN, L], f32)
        y_stack = big.tile([C_IN, L], f32r)

        # shared per-chunk LN-smalls: from psum mean/meansq to r and m rows
        def ln_smalls(ps_m, ps_e, w):
            m_c = sml.tile([1, CS], f32, tag="m_c", name="m_c")
            nc.scalar.copy(m_c[:, :w], ps_m[:, :w])
            m2_c = sml.tile([1, CS], f32, tag="m2_c", name="m2_c")
            nc.scalar.square(m2_c[:, :w], m_c[:, :w])
            var_c = sml.tile([1, CS], f32, tag="var_c", name="var_c")
            nc.vector.scalar_tensor_tensor(var_c[:, :w], ps_e[:, :w], EPS,
                                           m2_c[:, :w], OP.add, OP.subtract)
            lnv_c = sml.tile([1, CS], f32, tag="m2_c", name="lnv_c")
            nc.scalar.activation(lnv_c[:, :w], var_c[:, :w], AF.Ln)
            r_c = sml.tile([1, CS], f32, tag="r_c", name="r_c")
            nc.scalar.activation(r_c[:, :w], lnv_c[:, :w], AF.Exp, scale=-0.5)
            return r_c, m_c

        prev_sb = None
        s_t = None
        for i in range(HEAD):
            # ---- stage input s (32, L) at base partitions 0 ----
            chunk_sb = sml.tile([C_H, L], f32r, tag="s_cs", name="chunk_sb")
            nc.sync.dma_start(chunk_sb[:], x_shuf[32 * i:32 * (i + 1), :])
            if i == 0:
                s_t = chunk_sb[:]
            else:
                s_new = sml.tile([C_H, L], f32r, tag="s_cs", name="s_new")
                nc.vector.tensor_add(s_new[:], prev_sb[:],
                                     chunk_sb[:].bitcast(f32))
                s_t = s_new[:]

            # ---- LN1 + in_proj (LN applied post-matmul), chunked ----
            xzxx_sb = stg.tile([D, L], f32, tag="xzshare", name="xzxx_sb")
            z_sb = stg.tile([D, L], f32, tag="zg", name="z_sb")
            sz_sb = stg.tile([D, L], f32, tag="sz", name="sz_sb")
            for o, w in CHUNKS:
                sq_c = sml.tile([C_H, CS], f32r, tag="y2_c", name="sq_c")
                nc.vector.tensor_mul(sq_c[:, :w], s_t[:, o:o + w].bitcast(f32),
                                     s_t[:, o:o + w].bitcast(f32))
                ps_m = ps.tile([1, CS], f32, tag="sa", name="ps_m")
                ps_e = ps.tile([1, CS], f32, tag="sb", name="ps_e")
                nc.tensor.matmul(ps_m[:, :w], lhsT_st1[:], s_t[:, o:o + w],
                                 start=True, stop=True)
                nc.tensor.matmul(ps_e[:, :w], lhsT_st1[:], sq_c[:, :w],
                                 start=True, stop=True)
                r_c, m_c = ln_smalls(ps_m, ps_e, w)
                # r_rep broadcast to 64 rows
                ps_rr = ps.tile([D, CS], f32, tag="sc", name="ps_rr")
                nc.tensor.matmul(ps_rr[:, :w], lhsT_bc[0:1, 0:D], r_c[:, :w],
                                 start=True, stop=True)
                rr_c = sml.tile([D, CS], f32, tag="rr_c", name="rr_c")
                nc.scalar.copy(rr_c[:, :w], ps_rr[:, :w])
                # xx half
                ps_xx = ps.tile([D, CS], f32, tag="pd", name="ps_xx", bufs=2)
                nc.tensor.matmul(ps_xx[:, :w], lhsT_ip[:, 0:D],
                                 s_t[:, o:o + w], start=True, stop=False)
                nc.tensor.matmul(ps_xx[:, :w], lhsT_nw[:, 0:D],
                                 m_c[:, :w], start=False, stop=True)
                nc.vector.tensor_mul(xzxx_sb[:, o:o + w], ps_xx[:, :w],
                                     rr_c[:, :w])
                # z half -> silu
                ps_z = ps.tile([D, CS], f32, tag="pd", name="ps_z", bufs=2)
                nc.tensor.matmul(ps_z[:, :w], lhsT_ip[:, D:C_IN],
                                 s_t[:, o:o + w], start=True, stop=False)
                nc.tensor.matmul(ps_z[:, :w], lhsT_nw[:, D:C_IN],
                                 m_c[:, :w], start=False, stop=True)
                nc.vector.tensor_mul(z_sb[:, o:o + w], ps_z[:, :w],
                                     rr_c[:, :w])

            # xx into padded layout (+bias)
            nc.vector.tensor_scalar(xx_int, xzxx_sb[:], bias_xx[:], None, OP.add)

            # ---- depthwise 3x3 conv: 9-tap accumulate chain ----
            conv_acc = stg.tile([D, L], f32, tag="xzshare", name="conv_acc")
            xx_pv = xx_pad[:].rearrange("c (h w) -> c h w", h=50, w=50)
            first = True
            for dy in range(3):
                for dx in range(3):
                    tap = 3 * dy + dx
                    src_v = xx_pv[:, dy:dy + 48, dx:dx + 48]
                    ca = conv_acc[:].rearrange("c (h w) -> c h w", h=48, w=48)
                    if first:
                        nc.vector.tensor_scalar(ca, src_v, w9[:, tap:tap + 1],
                                                convb[:], OP.mult, OP.add)
                        first = False
                    else:
                        nc.vector.scalar_tensor_tensor(ca, src_v,
                                                       w9[:, tap:tap + 1],
                                                       ca, OP.mult, OP.add)

            # ---- silu + sequence orderings ----
            xs_s = stg.tile([D, L], bf16, tag="shared1b", name="xs_s")
            nc.scalar.activation(sz_sb[:], z_sb[:], AF.Silu, bias=bias_z[:])
            nc.scalar.activation(xs_s[:], conv_acc[:], AF.Silu)
            xs2_rm = stg.tile([C_IN, L], bf16, tag="xs2rm", name="xs2_rm")
            nc.sync.dma_start(xs2_rm[0:D, :], xs_s[:])
            nc.sync.dma_start(xs2_rm[D:C_IN, :], xs2_rm[0:D, :])
            xs2_cm = stg.tile([C_IN, L], bf16, tag="xs2cm", name="xs2_cm")
            nc.vector.tensor_copy(
                xs2_cm[0:D, :].rearrange("c (w h) -> c w h", h=48, w=48),
                xs2_rm[0:D, :].rearrange("c (h w) -> c w h", h=48, w=48))
            nc.sync.dma_start(xs2_cm[D:C_IN, :], xs2_cm[0:D, :])

            # ---- scan core, k order: rm (0,2) then cm (1,3) ----
            y_mid = stg.tile([D, L], f32, tag="shared1b", name="y_mid")
            nc.vector.tensor_scalar(y_mid[:], xs2_rm[0:D, :],
                                    Ds_q[:], None, OP.mult)
            y_cm_acc = stg.tile([D, L], f32, tag="ycm", name="y_cm_acc")
            y_cm_g = stg.tile([D, L], f32, tag="zg", name="y_cm_g")
            ar_cm_in = dram.tile([D, L], f32, tag="ar_cm_in", name="ar_cm_in")
            ar_cm_out = dram.tile([D, L], f32, tag="ar_cm_out", name="ar_cm_out")
            for k in (1, 3, 2, 0):
                xs2 = xs2_rm if k in (0, 2) else xs2_cm
                rev = k >= 2
                cls = "rm" if k in (0, 2) else "cm"

                h_prev = {0: None, 1: None}
                corder = list(range(len(SCHUNKS)))
                if rev:
                    corder = corder[::-1]
                for ci in corder:
                    o, w = SCHUNKS[ci]
                    # dt2 chunk = softplus(M2 @ xs + b), doubled halves
                    ps_dt = ps.tile([C_IN, SCS], f32, tag="sa", name="ps_dt")
                    for so in range(0, w, 512):
                        sw = min(512, w - so)
                        nc.tensor.matmul(ps_dt[:, so:so + sw], lhsT_M2[:, k, :],
                                         xs2[0:D, o + so:o + so + sw],
                                         start=True, stop=True)
                    e_ch = scn.tile([C_IN, SCS], f32, tag="u2_c", name="e_ch")
                    nc.scalar.activation(e_ch[:, :w], ps_dt[:, :w], AF.Exp,
                                         bias=dtb2[:, k:k + 1])
                    dt2_c = scn.tile([C_IN, SCS], f32, tag="dt2_c", name="dt2_c")
                    nc.scalar.activation(dt2_c[:, :w], e_ch[:, :w],
                                         AF.Ln, bias=1.0)
                    u2_c = scn.tile([C_IN, SCS], f32, tag="u2_c", name="u2_c")
                    nc.vector.tensor_mul(u2_c[:, :w], dt2_c[:, :w],
                                         xs2[:, o:o + w])
                    subs = [(so, min(512, w - so)) for so in range(0, w, 512)]
                    ps_ys = {}
                    for gl in range(2):
                        ps_b = ps.tile([C_IN, SCS], f32, tag="sb",
                                       name="ps_b")
                        ps_c = ps.tile([C_IN, SCS], f32, tag="sc",
                                       name="ps_c")
                        for so in range(0, w, 512):
                            sw = min(512, w - so)
                            nc.tensor.matmul(ps_b[:, so:so + sw],
                                             lhsT_B[:, k, gl, :],
                                             xs2[0:D, o + so:o + so + sw],
                                             start=True, stop=True)
                            nc.tensor.matmul(ps_c[:, so:so + sw],
                                             lhsT_C[:, k, gl, :],
                                             xs2[0:D, o + so:o + so + sw],
                                             start=True, stop=True)
                        dA = scn.tile([C_IN, SCS], f32, tag="dA", name="dA", bufs=1)
                        nc.scalar.activation(dA[:, :w], dt2_c[:, :w],
                                             AF.Exp, scale=A2[:, k, gl:gl + 1])
                        bB = scn.tile([C_IN, SCS], f32, tag="bB", name="bB")
                        nc.vector.tensor_mul(bB[:, :w], u2_c[:, :w],
                                             ps_b[:, :w])
                        h_c = hpool.tile([C_IN, SCS], f32, tag="h", name="h_c")
                        hp = h_prev[gl]
                        if not rev:
                            init = 0.0 if hp is None else hp[0][:, hp[1] - 1:hp[1]]
                            nc.vector.tensor_tensor_scan(
                                h_c[:, :w], dA[:, :w], bB[:, :w], init,
                                OP.mult, OP.add)
                        else:
                            init = 0.0 if hp is None else hp[0][:, 0:1]
                            nc.vector.tensor_tensor_scan(
                                h_c[:, :w][:, ::-1], dA[:, :w][:, ::-1],
                                bB[:, :w][:, ::-1], init, OP.mult, OP.add)
                        h_prev[gl] = (h_c, w)
                        hC = scn.tile([C_IN, SCS], bf16, tag="hC", name="hC")
                        nc.vector.tensor_mul(hC[:, :w], h_c[:, :w],
                                             ps_c[:, :w])
                        for so, sw in subs:
                            if gl == 0:
                                ps_ys[so] = ps.tile([D, 512], f32, tag="pd",
                                                    name="ps_y", bufs=2)
                            nc.tensor.matmul(ps_ys[so][:, :sw], lhsT_ys[:],
                                             hC[:, so:so + sw],
                                             start=(gl == 0), stop=(gl == 1),
                                             skip_group_check=True)
                    # drain sub-chunk y into SBUF accumulators
                    for so, sw in subs:
                        go = o + so
                        if k == 1:
                            nc.scalar.copy(y_cm_acc[:, go:go + sw],
                                           ps_ys[so][:, :sw])
                        elif k == 3:
                            nc.vector.tensor_add(y_cm_acc[:, go:go + sw],
                                                 y_cm_acc[:, go:go + sw],
                                                 ps_ys[so][:, :sw])
                        else:
                            nc.vector.tensor_add(y_mid[:, go:go + sw],
                                                 y_mid[:, go:go + sw],
                                                 ps_ys[so][:, :sw])
                    if k == 0 and ci in (1, len(SCHUNKS) - 1):
                        # rm part-ARs pipeline behind k=0's ascent
                        alo = 0 if ci == 1 else 2 * SCS
                        ahi = 2 * SCS if ci == 1 else L
                        aw = ahi - alo
                        ari = dram.tile([D, aw], f32, tag=f"ar_rm_in{ci}",
                                        name="ari", bufs=2)
                        aro = dram.tile([D, aw], f32, tag=f"ar_rm_out{ci}",
                                        name="aro", bufs=2)
                        nc.sync.dma_start(ari[:], y_mid[:, alo:ahi])
                        nc.gpsimd.collective_compute(
                            "AllReduce", OP.add,
                            replica_groups=[[0, 1, 2, 3], [4, 5, 6, 7]],
                            ins=[ari[:].opt()], outs=[aro[:].opt()])
                        nc.sync.dma_start(y_mid[:, alo:ahi], aro[:])
                if k == 3:
                    # cm pair complete: full AR + gather, hidden under rm
                    nc.sync.dma_start(ar_cm_in[:], y_cm_acc[:])
                    nc.gpsimd.collective_compute(
                        "AllReduce", OP.add,
                        replica_groups=[[0, 1, 2, 3], [4, 5, 6, 7]],
                        ins=[ar_cm_in[:].opt()], outs=[ar_cm_out[:].opt()])
                    nc.sync.dma_start(y_cm_acc[:], ar_cm_out[:])
                    nc.vector.tensor_copy(
                        y_cm_g[:].rearrange("c (h w) -> c h w", h=48, w=48),
                        y_cm_acc[:].rearrange("c (w h) -> c h w", h=48, w=48))

            # ---- out_norm LN (64 ch) + *silu(z) + out_proj + residual ----
            # per chunk: combine rm(AllReduced) + gathered cm, square, stats
            prev_new = stg.tile([C_H, L], f32, tag="prev", name="prev_new")
            for o, w in CHUNKS:
                nc.vector.tensor_add(y_stack[0:D, o:o + w], y_mid[:, o:o + w],
                                     y_cm_g[:, o:o + w])
                nc.sync.dma_start(y_stack[D:C_IN, o:o + w],
                                  y_stack[0:D, o:o + w])
                nc.vector.tensor_mul(y_stack[D:C_IN, o:o + w],
                                     y_stack[D:C_IN, o:o + w].bitcast(f32),
                                     y_stack[D:C_IN, o:o + w].bitcast(f32))
                ps_m = ps.tile([1, CS], f32, tag="sa", name="ps_m2")
                ps_e = ps.tile([1, CS], f32, tag="sb", name="ps_e2")
                nc.tensor.matmul(ps_m[:, :w], lhsT_st64[:, 0:1],
                                 y_stack[:, o:o + w], start=True, stop=True)
                nc.tensor.matmul(ps_e[:, :w], lhsT_st64[:, 1:2],
                                 y_stack[:, o:o + w], start=True, stop=True)
                r_c, m_c = ln_smalls(ps_m, ps_e, w)
                mr_c = sml.tile([1, CS], f32, tag="mr_c", name="mr_c", bufs=1)
                nc.vector.tensor_mul(mr_c[:, :w], m_c[:, :w], r_c[:, :w])
                ps_ra = ps.tile([D, CS], f32, tag="sc", name="ps_ra")
                nc.tensor.matmul(ps_ra[:, :w], lhsT_bc[0:1, 0:D], r_c[:, :w],
                                 start=True, stop=True)
                ps_rb = ps.tile([D, CS], f32, tag="pd", name="ps_rb", bufs=2)
                nc.tensor.matmul(ps_rb[:, :w], lhsT_bc[0:1, 0:D], mr_c[:, :w],
                                 start=True, stop=True)
                t1_c = sml.tile([D, CS], f32, tag="rr_c", name="t1_c")
                nc.vector.tensor_mul(t1_c[:, :w],
                                     y_stack[0:D, o:o + w].bitcast(f32),
                                     ps_ra[:, :w])
                nc.vector.tensor_sub(t1_c[:, :w], t1_c[:, :w], ps_rb[:, :w])
                y2_c = sml.tile([D, CS], f32r, tag="y2_c", name="y2_c")
                nc.vector.tensor_mul(y2_c[:, :w], t1_c[:, :w],
                                     sz_sb[:, o:o + w])
                ps_op = ps.tile([C_H, CS], f32, tag="pd", name="ps_op", bufs=2)
                nc.tensor.matmul(ps_op[:, :w], lhsT_op[:],
                                 y2_c[:, :w], start=True, stop=True)
                nc.vector.scalar_tensor_tensor(
                    prev_new[:, o:o + w], s_t[:, o:o + w].bitcast(f32),
                    1.0 + vs, ps_op[:, :w], OP.mult, OP.add)
            prev_sb = prev_new
            nc.sync.dma_start(outs_cat[32 * i:32 * (i + 1), :], prev_new[:])

        # ---- final: x_res = cvm*x_shuf + outs_cat; LN over 128 ch ----
        xres = big.tile([C_IN, L], f32r, tag="y_stack", name="xres")
        nc.vector.scalar_tensor_tensor(xres[:], x_shuf[:].bitcast(f32), cvm,
                                       outs_cat[:], OP.mult, OP.add)
        out_sb = big.tile([C_IN, L], f32, tag="outs_cat", name="out_sb")
        for o, w in CHUNKS:
            xsq_c = sml.tile([C_IN, CS], f32r, tag="xsq_c", name="xsq_c")
            nc.vector.tensor_mul(xsq_c[:, :w], xres[:, o:o + w].bitcast(f32),
                                 xres[:, o:o + w].bitcast(f32))
            ps_m = ps.tile([1, CS], f32, tag="sa", name="ps_m3")
            ps_e = ps.tile([1, CS], f32, tag="sb", name="ps_e3")
            nc.tensor.matmul(ps_m[:, :w], lhsT_st128[:, 0:1],
                             xres[:, o:o + w], start=True, stop=True)
            nc.tensor.matmul(ps_e[:, :w], lhsT_st128[:, 1:2],
                             xsq_c[:, :w], start=True, stop=True)
            r_c, m_c = ln_smalls(ps_m, ps_e, w)
            mr_c = sml.tile([1, CS], f32, tag="mr_c", name="mr_c3", bufs=1)
            nc.vector.tensor_mul(mr_c[:, :w], m_c[:, :w], r_c[:, :w])
            ps_ra = ps.tile([C_IN, CS], f32, tag="sc", name="ps_ra3")
            nc.tensor.matmul(ps_ra[:, :w], lhsT_bc[:], r_c[:, :w],
                             start=True, stop=True)
            ps_rb = ps.tile([C_IN, CS], f32, tag="pd", name="ps_rb3", bufs=2)
            nc.tensor.matmul(ps_rb[:, :w], lhsT_bc[:], mr_c[:, :w],
                             start=True, stop=True)
            nc.vector.tensor_mul(out_sb[:, o:o + w],
                                 xres[:, o:o + w].bitcast(f32), ps_ra[:, :w])
            nc.vector.tensor_sub(out_sb[:, o:o + w], out_sb[:, o:o + w],
                                 ps_rb[:, :w])
            nc.vector.tensor_scalar(out_sb[:, o:o + w], out_sb[:, o:o + w],
                                    gamma[:], beta[:], OP.mult, OP.add)
            nc.sync.dma_start(out_d[:, o:o + w], out_sb[:, o:o + w])

    nc.compile()
    return nc


def _host_prep(inputs):
    """Build per-core input maps from full inputs."""
    import ml_dtypes
    bf16 = ml_dtypes.bfloat16
    x = np.asarray(inputs["x"], np.float32)
    ln1_w = np.asarray(inputs["ln1_w"], np.float32)
    ln1_b = np.asarray(inputs["ln1_b"], np.float32)
    in_proj_w = np.asarray(inputs["in_proj_w"], np.float32)
    conv_w = np.asarray(inputs["conv_w"], np.float32)
    conv_b = np.asarray(inputs["conv_b"], np.float32)
    x_proj_w = np.asarray(inputs["x_proj_w"], np.float32)
    dt_proj_w = np.asarray(inputs["dt_proj_w"], np.float32)
    dt_proj_b = np.asarray(inputs["dt_proj_b"], np.float32)
    A_logs = np.asarray(inputs["A_logs"], np.float32)
    Ds = np.asarray(inputs["Ds"], np.float32)
    out_norm_w = np.asarray(inputs["out_norm_w"], np.float32)
    out_norm_b = np.asarray(inputs["out_norm_b"], np.float32)
    out_proj_w = np.asarray(inputs["out_proj_w"], np.float32)
    final_ln_w = np.asarray(inputs["final_ln_w"], np.float32)
    final_ln_b = np.asarray(inputs["final_ln_b"], np.float32)
    assert not np.any(out_norm_b), "out_norm_b must be zero (folded)"

    W_ip = (in_proj_w * ln1_w[None, :]).astype(np.float32)        # (128, 32)
    bias_ip = (in_proj_w @ ln1_b).astype(np.float32)              # (128,)
    lhsT_ip = np.ascontiguousarray(W_ip.T)                        # (32, 128)
    lhsT_nw = -W_ip.sum(1, keepdims=True).T.astype(np.float32)    # (1, 128)
    w9 = np.ascontiguousarray(
        conv_w[:, :, 0, :].transpose(2, 0, 1).reshape(D, 9))      # (64, 9)
    A = -np.exp(A_logs)                                           # (K, 64, 16)
    Ds_q = (Ds.sum(0) / 4.0).reshape(D, 1).astype(np.float32)
    W_op = (out_proj_w * out_norm_w[None, :]).astype(np.float32)  # (32, 64)
    lhsT_op = np.ascontiguousarray(W_op.T)                        # (64, 32)

    # dt fused projection, doubled: M2[d_in, k, p] = (dtw @ xpw_dt)[p%64, d_in]
    M = np.einsum("kdr,krc->kdc", dt_proj_w, x_proj_w[:, :DT_RANK, :])  # (K,64,64)
    lhsT_M2 = np.zeros((D, K, C_IN), bf16)
    for k in range(K):
        lhsT_M2[:, k, 0:D] = M[k].T
        lhsT_M2[:, k, D:C_IN] = M[k].T
    dtb2 = np.zeros((C_IN, K), np.float32)
    dtb2[0:D] = dt_proj_b.T
    dtb2[D:C_IN] = dt_proj_b.T

    # stats lhsTs
    lhsT_st1 = np.full((C_H, 1), 1.0 / C_H, np.float32)
    lhsT_bc = np.ones((1, C_IN), np.float32)
    lhsT_ys = np.zeros((C_IN, D), bf16)
    for p in range(C_IN):
        lhsT_ys[p, p % D] = 1.0
    lhsT_st64 = np.zeros((C_IN, 2), np.float32)
    lhsT_st64[0:D, 0] = 1.0 / D
    lhsT_st64[D:C_IN, 1] = 1.0 / D
    lhsT_st128 = np.zeros((C_IN, 2), np.float32)
    lhsT_st128[:, 0] = 1.0 / C_IN
    lhsT_st128[:, 1] = 1.0 / C_IN   # col1 used with xsq rhs

    common = {
        "lhsT_ip": lhsT_ip, "lhsT_nw": lhsT_nw,
        "bias_xx": bias_ip[0:D].reshape(D, 1),
        "bias_z": bias_ip[D:C_IN].reshape(D, 1),
        "w9": w9, "convb": conv_b.reshape(D, 1),
        "lhsT_st1": lhsT_st1, "lhsT_bc": lhsT_bc,
        "lhsT_M2": lhsT_M2, "dtb2": dtb2,
        "Ds_q": Ds_q, "lhsT_ys": lhsT_ys,
        "lhsT_st64": lhsT_st64, "lhsT_op": lhsT_op,
        "lhsT_st128": lhsT_st128,
        "gamma": final_ln_w.reshape(C_IN, 1),
        "beta": final_ln_b.reshape(C_IN, 1),
    }

    # per-b shuffled channels-first inputs
    g, cg = HEAD, C_IN // HEAD
    per_b = []
    for b in range(B):
        xs = x[b].reshape(H, W, g, cg).transpose(0, 1, 3, 2).reshape(L, C_IN)
        x_shuf_cf = np.ascontiguousarray(xs.T)  # (128, L)
        per_b.append(x_shuf_cf)

    in_maps = []
    for c in range(NCORES):
        b, nh = c // 4, c % 4
        # per-core A2 / B / C expanded weights for n slice [4nh, 4nh+4)
        A2 = np.zeros((C_IN, K, 2), np.float32)
        lhsT_B = np.zeros((D, K, 2, C_IN), bf16)
        lhsT_C = np.zeros((D, K, 2, C_IN), bf16)
        for k in range(K):
            for gl in range(2):
                for half in range(2):
                    n = 4 * nh + 2 * gl + half
                    rows = slice(64 * half, 64 * half + 64)
                    A2[rows, k, gl] = A[k, :, n]
                    lhsT_B[:, k, gl, rows] = x_proj_w[k, DT_RANK + n, :][:, None]
                    lhsT_C[:, k, gl, rows] = x_proj_w[k, DT_RANK + N + n, :][:, None]
        x_shuf_cf = per_b[b]
        in_maps.append(dict(common, x_shuf=x_shuf_cf,
                            A2=A2, lhsT_B=lhsT_B, lhsT_C=lhsT_C))
    vs = float(np.asarray(inputs["vss_skip"]).ravel()[0])
    cvm = float(np.asarray(inputs["cvm_skip"]).ravel()[0])
    return in_maps, vs, cvm


def kernel(**inputs) -> np.ndarray:
    from concourse.bass_utils import run_bass_kernel_spmd

    in_maps, vs, cvm = _host_prep(inputs)
    key = (vs, cvm)
    if key not in _cache:
        _cache[key] = _build(vs, cvm)
    nc = _cache[key]
    res = run_bass_kernel_spmd(nc, in_maps, core_ids=list(range(NCORES)))
    out = np.zeros((B, H, W, C_IN), np.float32)
    for b in range(B):
        out_cf = res.results[4 * b]["out_cf"]  # (128, L)
        out[b] = out_cf.T.reshape(H, W, C_IN)
    return out



# revision 12
# speedup vs baseline: 1.3666x; 1.3666x over previous
"""Trainium2 Bass kernel for nn_CascadedVMambaBlock.

Sharding: 8 cores; core c = (b, nh) with b = c//4, nh = c%4.
Each core processes sample b with state-dim slice n in [4nh, 4nh+4)
for ALL 4 scan directions k; one bf16 AllReduce per stage over each
4-core b-group combines the n-partials of y (cm pair folded in via a
transposed add, Ds term via a diagonal matmul into the same PSUM
accumulation group).

Everything on-device is channels-first: (channels on partitions,
L = 48*48 = 2304 pixels on the free dim). All matmuls use bf16
operands (1-pass); the depthwise 3x3 conv runs on the PE as 9
diagonal-lhsT matmuls accumulating in PSUM. Per-pixel LN scalars
(r, m*r) are broadcast across partitions with 1-row/2-row bf16
matmuls. Two tiny warm-up AllReduces at kernel start absorb the
cross-core skew that otherwise stalls the first real collective.

Per (k, gl) scan-lane layout (gl in {0,1}): partition p = half*64 + d
covers state n = 4*nh + 2*gl + half, channel d = p % 64.
The selective-scan recurrence runs as tensor_tensor_scan along the
free dim; k in {2,3} scan backwards via negative-step APs.
"""
import numpy as np

# problem constants (hardcoded; kernel.py must be self-contained)
HEAD, C_IN, C_H = 4, 128, 32
D, N, K, DT_RANK = 64, 16, 4, 2
B, H, W = 2, 48, 48
L = H * W            # 2304
EPS = 1e-5
NCORES = 8

SCS = 1008           # scan chunk size along L (21 image rows)
SCHUNKS = [(0, 1008), (1008, 1008), (2016, 288)]
MMW = 512            # matmul window (PSUM bank limit)
# ln1 passes: row-aligned (20/20/8 rows); windows of 480 stay row-aligned
LN1P = [(0, 960), (960, 960), (1920, 384)]
LNW = 480
# conv row chunks; chunk (r0, nr) needs xx rows [r0-1, r0+nr] available
CONVR = [(0, 10), (10, 9), (19, 10), (29, 10), (39, 9)]

_cache = {}


def _win(w, step=MMW):
    return [(so, min(step, w - so)) for so in range(0, w, step)]


def _build(vs, cvm):
    import concourse.bass as bass
    import concourse.bacc as bacc
    import concourse.tile as tile
    import concourse.mybir as mybir
    from contextlib import ExitStack

    f32 = mybir.dt.float32
    bf16 = mybir.dt.bfloat16
    AF = mybir.ActivationFunctionType
    OP = mybir.AluOpType

    # Force the ACT-table chooser to a single exp/ln table (+ the silu
    # table): avoids ACT_TABLE_LOAD thrash between Ln/Exp variants.
    import concourse.hw_specs as hw_specs
    _orig_gat = hw_specs.get_activation_tables
    _KEEP = {"natural_log_exp_and_others", "silu_and_others"}

    def _patched_gat(arch):
        t = _orig_gat(arch)
        return {k: (v if k in _KEEP else set()) for k, v in t.items()}

    bacc.get_activation_tables = _patched_gat

    nc = bacc.Bacc("TRN2", target_bir_lowering=False, debug=False,
                   enable_asserts=True, num_devices=NCORES)

    def din(name, shape, dtype=f32):
        return nc.dram_tensor(name, shape, dtype, kind="ExternalInput").ap()

    x_shuf_d = din("x_shuf", (C_IN, L), bf16)         # shuffled input bf16
    lhsT_ip2_d = din("lhsT_ip2", (C_H, C_IN), bf16)   # in_proj lhsT (32,128)
    lhsT_nw2_d = din("lhsT_nw2", (1, C_IN), bf16)     # -colsum(W_ip) row
    bias_xx_d = din("bias_xx", (D, 1))                # in_proj bias rows 0:64
    bias_z_d = din("bias_z", (D, 1))                  # rows 64:128
    w9d_d = din("w9d", (D, 9, D), bf16)               # conv diag lhsT per tap
    convb_d = din("convb", (D, 1))
    lhsT_st2_d = din("lhsT_st2", (D, 33), bf16)        # ln1 stats mask
    lhsT_bc128_d = din("lhsT_bc128", (1, C_IN), bf16)  # ones bcast row
    lhsT_rm2_d = din("lhsT_rm2", (33, C_IN), bf16)     # out_norm r/mr bcast
    lhsT_bcg_d = din("lhsT_bcg", (1, C_IN), bf16)     # final gamma-bcast row
    lhsT_M2_d = din("lhsT_M2", (D, K, C_IN), bf16)    # dt fused proj, doubled
    dtb2_d = din("dtb2", (C_IN, K))                   # dt bias doubled
    A2_d = din("A2", (C_IN, K, 2))                    # A scale per (k, gl)
    lhsT_B_d = din("lhsT_B", (D, K, 2, C_IN), bf16)   # B_exp weights
    lhsT_C_d = din("lhsT_C", (D, K, 2, C_IN), bf16)   # C_exp weights
    lhsT_ys_d = din("lhsT_ys", (C_IN, D), bf16)       # half-sum (p%64==d)
    lhsT_Dsq_d = din("lhsT_Dsq", (D, D), bf16)        # diag(sum_k Ds / 4)
    lhsT_st64_d = din("lhsT_st64", (C_IN, 33), bf16)   # out_norm stats mask
    lhsT_op2_d = din("lhsT_op2", (D, C_H), bf16)      # out_proj lhsT
    lhsT_s128_d = din("lhsT_s128", (C_IN, 1), bf16)   # final mean mask
    gamma_d = din("gamma", (C_IN, 1))
    beta_d = din("beta", (C_IN, 1))

    out_d = nc.dram_tensor("out_cf", (C_IN, L), f32, kind="ExternalOutput").ap()

    with tile.TileContext(nc) as tc, ExitStack() as ctx:
        w_pool = ctx.enter_context(tc.tile_pool(name="weights", bufs=1))
        big = ctx.enter_context(tc.tile_pool(name="big", bufs=1))
        stg = ctx.enter_context(tc.tile_pool(name="stg", bufs=1))
        sml = ctx.enter_context(tc.tile_pool(name="sml", bufs=2))
        scn = ctx.enter_context(tc.tile_pool(name="scn", bufs=2))
        hpool = ctx.enter_context(tc.tile_pool(name="hpool", bufs=5))
        ps = ctx.enter_context(tc.tile_pool(name="ps", bufs=1, space="PSUM"))
        dram = ctx.enter_context(tc.tile_pool(name="dram", bufs=2, space="DRAM"))

        def wload(ap_d, shape, dtype=f32):
            t = w_pool.tile(list(shape), dtype, name=ap_d.tensor.name + "_sb")
            src = ap_d if ap_d.dtype == dtype else ap_d.bitcast(dtype)
            nc.sync.dma_start(t[:], src)
            return t

        x_shuf = wload(x_shuf_d, (C_IN, L), bf16)
        lhsT_ip2 = wload(lhsT_ip2_d, (C_H, C_IN), bf16)
        lhsT_nw2 = wload(lhsT_nw2_d, (1, C_IN), bf16)
        bias_xx = wload(bias_xx_d, (D, 1))
        bias_z = wload(bias_z_d, (D, 1))
        w9d = wload(w9d_d, (D, 9, D), bf16)
        convb = wload(convb_d, (D, 1))
        lhsT_st2 = wload(lhsT_st2_d, (D, 33), bf16)
        lhsT_bc128 = wload(lhsT_bc128_d, (1, C_IN), bf16)
        lhsT_rm2 = wload(lhsT_rm2_d, (33, C_IN), bf16)
        lhsT_bcg = wload(lhsT_bcg_d, (1, C_IN), bf16)
        lhsT_M2 = wload(lhsT_M2_d, (D, K, C_IN), bf16)
        dtb2 = wload(dtb2_d, (C_IN, K))
        A2 = wload(A2_d, (C_IN, K, 2))
        lhsT_B = wload(lhsT_B_d, (D, K, 2, C_IN), bf16)
        lhsT_C = wload(lhsT_C_d, (D, K, 2, C_IN), bf16)
        lhsT_ys = wload(lhsT_ys_d, (C_IN, D), bf16)
        lhsT_Dsq = wload(lhsT_Dsq_d, (D, D), bf16)
        lhsT_st64 = wload(lhsT_st64_d, (C_IN, 33), bf16)
        lhsT_op2 = wload(lhsT_op2_d, (D, C_H), bf16)
        lhsT_s128 = wload(lhsT_s128_d, (C_IN, 1), bf16)
        gamma = wload(gamma_d, (C_IN, 1))
        beta = wload(beta_d, (C_IN, 1))

        # warm-up collectives: absorb cross-core start skew while the
        # front-end computes. Two back-to-back tiny AllReduces.
        for wi in range(2):
            wu_in = dram.tile([1, 16], f32, tag=f"wu_in{wi}", name="wu_in")
            wu_out = dram.tile([1, 16], f32, tag=f"wu_out{wi}", name="wu_out")
            nc.gpsimd.collective_compute(
                "AllReduce", OP.add,
                replica_groups=[[0, 1, 2, 3], [4, 5, 6, 7]],
                ins=[wu_in[:].opt()], outs=[wu_out[:].opt()])

        # persistent big tensors
        xx_pad = big.tile([D, 50 * 50], bf16)       # zero-padded conv input
        nc.vector.memset(xx_pad[:], 0.0)
        xx_pv = xx_pad[:].rearrange("c (h w) -> c h w", h=50, w=50)
        outs_cat = big.tile([C_IN, L], bf16)

        prev_sb = None
        for i in range(HEAD):
            # ---- stage input s (bf16) + squares tile (64, L) ----
            st = stg.tile([D, L], bf16, tag="st", name="st")
            if i == 0:
                nc.sync.dma_start(st[0:C_H, :], x_shuf[0:C_H, :])
            else:
                chunk_sb = stg.tile([C_H, L], bf16, tag="chunk", name="chunk_sb")
                nc.sync.dma_start(chunk_sb[:], x_shuf[C_H * i:C_H * (i + 1), :])
                nc.vector.tensor_add(st[0:C_H, :], prev_sb[:], chunk_sb[:])
            s_t = st[0:C_H, :]

            # ---- LN1 + in_proj (3 row-aligned passes) ----
            z_pre = stg.tile([D, L], bf16, tag="zpre", name="z_pre")
            for (o, w) in LN1P:
                nc.scalar.square(st[C_H:D, o:o + w], st[0:C_H, o:o + w])
                for so, sw in _win(w, LNW):
                    go = o + so
                    ps_st = ps.tile([33, MMW], f32, tag="sa", name="ps_st",
                                    bufs=2)
                    nc.tensor.matmul(ps_st[:, :sw], lhsT_st2[:],
                                     st[0:D, go:go + sw], start=True, stop=True)
                    m_c = sml.tile([1, MMW], bf16, tag="m_c", name="m_c")
                    nc.vector.tensor_copy(m_c[:, :sw], ps_st[0:1, :sw])
                    m2_c = sml.tile([1, MMW], bf16, tag="m2_c", name="m2_c")
                    nc.vector.tensor_mul(m2_c[:, :sw], m_c[:, :sw], m_c[:, :sw])
                    var_c = sml.tile([1, MMW], f32, tag="var_c", name="var_c")
                    nc.vector.scalar_tensor_tensor(
                        var_c[:, :sw], ps_st[32:33, :sw], EPS, m2_c[:, :sw],
                        OP.add, OP.subtract)
                    lnv_c = sml.tile([1, MMW], f32, tag="lnv_c", name="lnv_c")
                    nc.scalar.activation(lnv_c[:, :sw], var_c[:, :sw], AF.Ln)
                    r_c = sml.tile([1, MMW], bf16, tag="r_c", name="r_c")
                    nc.scalar.activation(r_c[:, :sw], lnv_c[:, :sw], AF.Exp,
                                         scale=-0.5)
                    ps_rr = ps.tile([C_IN, MMW], f32, tag="rm", name="ps_rr")
                    nc.tensor.matmul(ps_rr[:, :sw], lhsT_bc128[:], r_c[:, :sw],
                                     start=True, stop=True)
                    rr_sb = sml.tile([C_IN, MMW], bf16, tag="rr", name="rr_sb")
                    nc.scalar.copy(rr_sb[:, :sw], ps_rr[:, :sw])
                    ps_xz = ps.tile([C_IN, MMW], f32, tag="dt", name="ps_xz")
                    nc.tensor.matmul(ps_xz[:, :sw], lhsT_ip2[:],
                                     st[0:C_H, go:go + sw], start=True,
                                     stop=False)
                    nc.tensor.matmul(ps_xz[:, :sw], lhsT_nw2[:],
                                     m_c[:, :sw], start=False, stop=True)
                    # xx half -> padded conv layout (bf16), z half -> z_pre
                    r0 = go // W
                    nr = sw // W
                    xx_dst = xx_pv[:, 1 + r0:1 + r0 + nr, 1:49]
                    nc.vector.tensor_mul(
                        xx_dst,
                        ps_xz[0:D, :sw].rearrange("c (h w) -> c h w", h=nr, w=W),
                        rr_sb[0:D, :sw].rearrange("c (h w) -> c h w", h=nr, w=W))
                    nc.vector.tensor_scalar(xx_dst, xx_dst, bias_xx[:], None,
                                            OP.add)
                    nc.vector.tensor_mul(z_pre[:, go:go + sw],
                                         ps_xz[D:C_IN, :sw], rr_sb[D:C_IN, :sw])
            sz = stg.tile([D, L], bf16, tag="sz", name="sz")
            nc.scalar.activation(sz[:], z_pre[:], AF.Silu, bias=bias_z[:])

            # ---- depthwise 3x3 conv on PE: 9 diag matmuls per row-chunk ----
            xs2_rm = stg.tile([C_IN, L], bf16, tag="xs2rm", name="xs2_rm")
            for (r0, nr) in CONVR:
                pw = nr * W
                ps_cv = ps.tile([D, MMW], f32, tag="ys", name="ps_cv", bufs=2)
                for tap in range(9):
                    dy, dx = tap // 3, tap % 3
                    rhs = xx_pv[:, dy + r0:dy + r0 + nr, dx:dx + W]
                    nc.tensor.matmul(ps_cv[:, :pw], w9d[:, tap, :], rhs,
                                     start=(tap == 0), stop=(tap == 8))
                nc.scalar.activation(xs2_rm[0:D, r0 * W:r0 * W + pw],
                                     ps_cv[:, :pw], AF.Silu, bias=convb[:])
            nc.sync.dma_start(xs2_rm[D:C_IN, :], xs2_rm[0:D, :])
            xs2_cm = stg.tile([C_IN, L], bf16, tag="xs2cm", name="xs2_cm")
            nc.vector.tensor_copy(
                xs2_cm[0:D, :].rearrange("c (w h) -> c w h", h=48, w=48),
                xs2_rm[0:D, :].rearrange("c (h w) -> c w h", h=48, w=48))
            nc.sync.dma_start(xs2_cm[D:C_IN, :], xs2_cm[0:D, :])

            # ---- scan core, k order: cm (1,3) then rm (2,0) ----
            y_cm = stg.tile([D, L], f32, tag="ycm", name="y_cm")
            y_full = stg.tile([D, L], f32, tag="yfull", name="y_full")
            y_stack = stg.tile([C_IN, L], bf16, tag="ystack", name="y_stack")
            y_cmT = y_cm[:].rearrange("c (w h) -> c h w", w=48, h=48)

            for k in (1, 3, 2, 0):
                xs2 = xs2_rm if k in (0, 2) else xs2_cm
                rev = k >= 2

                h_prev = {0: None, 1: None}
                corder = list(range(len(SCHUNKS)))
                if rev:
                    corder = corder[::-1]
                for ci in corder:
                    o, w = SCHUNKS[ci]
                    subs = _win(w)
                    # dt path (doubled halves): e = exp(raw+b), dt2 = ln(1+e)
                    ps_dt = ps.tile([C_IN, MMW], f32, tag="dt", name="ps_dt")
                    e_ch = scn.tile([C_IN, SCS], f32, tag="e_ch", name="e_ch")
                    for so, sw in subs:
                        nc.tensor.matmul(ps_dt[:, :sw], lhsT_M2[:, k, :],
                                         xs2[0:D, o + so:o + so + sw],
                                         start=True, stop=True)
                        nc.scalar.activation(e_ch[:, so:so + sw],
                                             ps_dt[:, :sw], AF.Exp,
                                             bias=dtb2[:, k:k + 1])
                    dt2_c = scn.tile([C_IN, SCS], f32, tag="dt2", name="dt2_c")
                    nc.scalar.activation(dt2_c[:, :w], e_ch[:, :w],
                                         AF.Ln, bias=1.0)
                    u2_c = scn.tile([C_IN, SCS], bf16, tag="u2", name="u2_c")
                    nc.gpsimd.tensor_mul(u2_c[:, :w], dt2_c[:, :w],
                                         xs2[:, o:o + w])
                    ps_ys = None
                    for gl in range(2):
                        ps_b = ps.tile([C_IN, MMW], f32, tag="b", name="ps_b")
                        bB = scn.tile([C_IN, SCS], f32, tag="bB", name="bB")
                        for so, sw in subs:
                            nc.tensor.matmul(ps_b[:, :sw], lhsT_B[:, k, gl, :],
                                             xs2[0:D, o + so:o + so + sw],
                                             start=True, stop=True)
                            nc.vector.tensor_mul(bB[:, so:so + sw],
                                                 u2_c[:, so:so + sw],
                                                 ps_b[:, :sw])
                        dA = scn.tile([C_IN, SCS], f32, tag="dA", name="dA")
                        nc.scalar.activation(dA[:, :w], dt2_c[:, :w],
                                             AF.Exp, scale=A2[:, k, gl:gl + 1])
                        h_c = hpool.tile([C_IN, SCS], f32, tag="h", name="h_c")
                        hp = h_prev[gl]
                        if not rev:
                            init = 0.0 if hp is None else hp[0][:, hp[1] - 1:hp[1]]
                            nc.vector.tensor_tensor_scan(
                                h_c[:, :w], dA[:, :w], bB[:, :w], init,
                                OP.mult, OP.add)
                        else:
                            init = 0.0 if hp is None else hp[0][:, 0:1]
                            nc.vector.tensor_tensor_scan(
                                h_c[:, :w][:, ::-1], dA[:, :w][:, ::-1],
                                bB[:, :w][:, ::-1], init, OP.mult, OP.add)
                        h_prev[gl] = (h_c, w)
                        hC = scn.tile([C_IN, SCS], bf16, tag="hC", name="hC")
                        for so, sw in subs:
                            ps_c = ps.tile([C_IN, MMW], f32, tag="c",
                                           name="ps_c")
                            nc.tensor.matmul(ps_c[:, :sw], lhsT_C[:, k, gl, :],
                                             xs2[0:D, o + so:o + so + sw],
                                             start=True, stop=True)
                            nc.vector.tensor_mul(hC[:, so:so + sw],
                                                 h_c[:, so:so + sw],
                                                 ps_c[:, :sw])
                        if gl == 0:
                            ps_ys = [ps.tile([D, MMW], f32, tag="ys",
                                             name="ps_ys", bufs=2)
                                     for _ in subs]
                            if k == 2:
                                # Ds term joins the k=2 accumulation group
                                for si, (so, sw) in enumerate(subs):
                                    nc.tensor.matmul(
                                        ps_ys[si][:, :sw], lhsT_Dsq[:],
                                        xs2_rm[0:D, o + so:o + so + sw],
                                        start=True, stop=False,
                                        skip_group_check=True)
                        start_flag = (gl == 0 and k != 2)
                        for si, (so, sw) in enumerate(subs):
                            nc.tensor.matmul(ps_ys[si][:, :sw], lhsT_ys[:],
                                             hC[:, so:so + sw],
                                             start=start_flag,
                                             stop=(gl == 1),
                                             skip_group_check=True)
                    # drain: k1/k2 -> ACT copy, k3/k0 -> DVE add
                    ydst = y_cm if k in (1, 3) else y_full
                    for si, (so, sw) in enumerate(subs):
                        go = o + so
                        if k == 1 or k == 2:
                            nc.scalar.copy(ydst[:, go:go + sw],
                                           ps_ys[si][:, :sw])
                        else:
                            nc.vector.tensor_add(ydst[:, go:go + sw],
                                                 ydst[:, go:go + sw],
                                                 ps_ys[si][:, :sw])
                    if k == 0:
                        # rm chunk complete: fold cm (transposed) and
                        # AllReduce this chunk in bf16 behind later scans
                        r0 = o // W
                        nr = w // W
                        yt = sml.tile([D, SCS], bf16, tag="yt", name="yt")
                        nc.vector.tensor_add(
                            yt[:, :w].rearrange("c (h w) -> c h w", h=nr, w=W),
                            y_full[:, o:o + w].rearrange(
                                "c (h w) -> c h w", h=nr, w=W),
                            y_cmT[:, r0:r0 + nr, :])
                        ar_i = dram.tile([D, w], bf16, tag=f"ar_i{ci}",
                                         name="ar_i")
                        ar_o = dram.tile([D, w], bf16, tag=f"ar_o{ci}",
                                         name="ar_o")
                        nc.sync.dma_start(ar_i[:], yt[:, :w])
                        nc.gpsimd.collective_compute(
                            "AllReduce", OP.add,
                            replica_groups=[[0, 1, 2, 3], [4, 5, 6, 7]],
                            ins=[ar_i[:].opt()], outs=[ar_o[:].opt()])
                        nc.sync.dma_start(y_stack[0:D, o:o + w], ar_o[:])

            # ---- out_norm LN + gate + out_proj + residual, per AR chunk ----
            prev_new = stg.tile([C_H, L], bf16, tag="prev", name="prev_new")
            for (o, w) in SCHUNKS:
                nc.scalar.square(y_stack[D:C_IN, o:o + w],
                                 y_stack[0:D, o:o + w])
                ps_st64 = ps.tile([33, MMW], f32, tag="sa", name="ps_st64",
                                  bufs=2)
                rm2 = sml.tile([33, MMW], bf16, tag="rm2", name="rm2")
                for so, sw in _win(w):
                    go = o + so
                    nc.tensor.matmul(ps_st64[:, :sw], lhsT_st64[:],
                                     y_stack[:, go:go + sw],
                                     start=True, stop=True)
                    m_c = sml.tile([1, MMW], bf16, tag="m_c", name="m_c2")
                    nc.vector.tensor_copy(m_c[:, :sw], ps_st64[0:1, :sw])
                    m2_c = sml.tile([1, MMW], bf16, tag="m2_c", name="m2_c2")
                    nc.vector.tensor_mul(m2_c[:, :sw], m_c[:, :sw], m_c[:, :sw])
                    var_c = sml.tile([1, MMW], f32, tag="var_c", name="var_c2")
                    nc.vector.scalar_tensor_tensor(
                        var_c[:, :sw], ps_st64[32:33, :sw], EPS, m2_c[:, :sw],
                        OP.add, OP.subtract)
                    lnv_c = sml.tile([1, MMW], f32, tag="lnv_c", name="lnv_c2")
                    nc.scalar.activation(lnv_c[:, :sw], var_c[:, :sw], AF.Ln)
                    nc.scalar.activation(rm2[0:1, :sw], lnv_c[:, :sw], AF.Exp,
                                         scale=-0.5)
                    nc.vector.tensor_mul(rm2[32:33, :sw], m_c[:, :sw],
                                         rm2[0:1, :sw])
                    ps_rm = ps.tile([C_IN, MMW], f32, tag="rm", name="ps_rm")
                    nc.tensor.matmul(ps_rm[:, :sw], lhsT_rm2[:], rm2[:, :sw],
                                     start=True, stop=True)
                    t1 = sml.tile([D, MMW], f32, tag="t1", name="t1")
                    nc.vector.tensor_mul(t1[:, :sw], y_stack[0:D, go:go + sw],
                                         ps_rm[0:D, :sw])
                    nc.vector.tensor_sub(t1[:, :sw], t1[:, :sw],
                                         ps_rm[D:C_IN, :sw])
                    y2 = sml.tile([D, MMW], bf16, tag="y2", name="y2")
                    nc.vector.tensor_mul(y2[:, :sw], t1[:, :sw],
                                         sz[:, go:go + sw])
                    ps_op = ps.tile([C_H, MMW], f32, tag="b", name="ps_op")
                    nc.tensor.matmul(ps_op[:, :sw], lhsT_op2[:], y2[:, :sw],
                                     start=True, stop=True)
                    nc.vector.scalar_tensor_tensor(
                        prev_new[:, go:go + sw], st[0:C_H, go:go + sw],
                        1.0 + vs, ps_op[:, :sw], OP.mult, OP.add)
            prev_sb = prev_new
            nc.sync.dma_start(outs_cat[C_H * i:C_H * (i + 1), :], prev_new[:])

        # ---- final: x_res = cvm*x_shuf + outs_cat; LN over 128 ch ----
        xres = big.tile([C_IN, L], bf16, tag="xres", name="xres")
        xsq = big.tile([C_IN, L], bf16, tag="xsq", name="xsq")
        out_sb = stg.tile([C_IN, L], f32, tag="outsb", name="out_sb")
        for (o, w) in SCHUNKS:
            nc.vector.scalar_tensor_tensor(xres[:, o:o + w],
                                           x_shuf[:, o:o + w], cvm,
                                           outs_cat[:, o:o + w],
                                           OP.mult, OP.add)
            nc.scalar.square(xsq[:, o:o + w], xres[:, o:o + w])
            for so, sw in _win(w):
                go = o + so
                ps_sf = ps.tile([33, MMW], f32, tag="sa", name="ps_sf", bufs=2)
                nc.tensor.matmul(ps_sf[0:1, :sw], lhsT_s128[:],
                                 xres[:, go:go + sw], start=True, stop=True)
                nc.tensor.matmul(ps_sf[32:33, :sw], lhsT_s128[:],
                                 xsq[:, go:go + sw], start=True, stop=True)
                m_c = sml.tile([1, MMW], bf16, tag="m_c", name="m_c3")
                nc.vector.tensor_copy(m_c[:, :sw], ps_sf[0:1, :sw])
                m2_c = sml.tile([1, MMW], bf16, tag="m2_c", name="m2_c3")
                nc.vector.tensor_mul(m2_c[:, :sw], m_c[:, :sw], m_c[:, :sw])
                var_c = sml.tile([1, MMW], f32, tag="var_c", name="var_c3")
                nc.vector.scalar_tensor_tensor(
                    var_c[:, :sw], ps_sf[32:33, :sw], EPS, m2_c[:, :sw],
                    OP.add, OP.subtract)
                lnv_c = sml.tile([1, MMW], f32, tag="lnv_c", name="lnv_c3")
                nc.scalar.activation(lnv_c[:, :sw], var_c[:, :sw], AF.Ln)
                r_c = sml.tile([1, MMW], bf16, tag="r_c", name="r_c3")
                nc.scalar.activation(r_c[:, :sw], lnv_c[:, :sw], AF.Exp,
                                     scale=-0.5)
                mr_c = sml.tile([1, MMW], bf16, tag="mr_c", name="mr_c3")
                nc.vector.tensor_mul(mr_c[:, :sw], m_c[:, :sw], r_c[:, :sw])
                ps_ra = ps.tile([C_IN, MMW], f32, tag="rm", name="ps_ra")
                nc.tensor.matmul(ps_ra[:, :sw], lhsT_bc128[:], r_c[:, :sw],
                                 start=True, stop=True)
                ps_rb = ps.tile([C_IN, MMW], f32, tag="dt", name="ps_rb")
                nc.tensor.matmul(ps_rb[:, :sw], lhsT_bcg[:], mr_c[:, :sw],
                                 start=True, stop=True)
                t1 = sml.tile([C_IN, MMW], f32, tag="ft1", name="ft1")
                nc.vector.tensor_mul(t1[:, :sw], xres[:, go:go + sw],
                                     ps_ra[:, :sw])
                nc.vector.tensor_scalar(t1[:, :sw], t1[:, :sw], gamma[:],
                                        beta[:], OP.mult, OP.add)
                nc.vector.tensor_sub(out_sb[:, go:go + sw], t1[:, :sw],
                                     ps_rb[:, :sw])
                nc.sync.dma_start(out_d[:, go:go + sw],
                                  out_sb[:, go:go + sw])

    nc.compile()
    return nc


def _host_prep(inputs):
    """Build per-core input maps from full inputs."""
    import ml_dtypes
    bf16 = ml_dtypes.bfloat16
    x = np.asarray(inputs["x"], np.float32)
    ln1_w = np.asarray(inputs["ln1_w"], np.float32)
    ln1_b = np.asarray(inputs["ln1_b"], np.float32)
    in_proj_w = np.asarray(inputs["in_proj_w"], np.float32)
    conv_w = np.asarray(inputs["conv_w"], np.float32)
    conv_b = np.asarray(inputs["conv_b"], np.float32)
    x_proj_w = np.asarray(inputs["x_proj_w"], np.float32)
    dt_proj_w = np.asarray(inputs["dt_proj_w"], np.float32)
    dt_proj_b = np.asarray(inputs["dt_proj_b"], np.float32)
    A_logs = np.asarray(inputs["A_logs"], np.float32)
    Ds = np.asarray(inputs["Ds"], np.float32)
    out_norm_w = np.asarray(inputs["out_norm_w"], np.float32)
    out_norm_b = np.asarray(inputs["out_norm_b"], np.float32)
    out_proj_w = np.asarray(inputs["out_proj_w"], np.float32)
    final_ln_w = np.asarray(inputs["final_ln_w"], np.float32)
    final_ln_b = np.asarray(inputs["final_ln_b"], np.float32)
    assert not np.any(out_norm_b), "out_norm_b must be zero (folded)"

    W_ip = (in_proj_w * ln1_w[None, :]).astype(np.float32)        # (128, 32)
    bias_ip = (in_proj_w @ ln1_b).astype(np.float32)              # (128,)
    lhsT_ip2 = np.ascontiguousarray(W_ip.T).astype(bf16)          # (32, 128)
    lhsT_nw2 = (-W_ip.sum(1, keepdims=True).T).astype(bf16)       # (1, 128)
    # conv diag lhsT per tap: (64, 9, 64)
    w9 = conv_w[:, :, 0, :].transpose(2, 0, 1).reshape(D, 9)      # (64, 9)
    w9d = np.zeros((D, 9, D), bf16)
    for t in range(9):
        w9d[np.arange(D), t, np.arange(D)] = w9[:, t].astype(bf16)
    A = -np.exp(A_logs)                                           # (K, 64, 16)
    Ds_q = (Ds.sum(0) / 4.0).astype(np.float32)                   # (64,)
    lhsT_Dsq = np.zeros((D, D), bf16)
    lhsT_Dsq[np.arange(D), np.arange(D)] = Ds_q.astype(bf16)
    W_op = (out_proj_w * out_norm_w[None, :]).astype(np.float32)  # (32, 64)
    lhsT_op2 = np.ascontiguousarray(W_op.T).astype(bf16)          # (64, 32)

    M = np.einsum("kdr,krc->kdc", dt_proj_w, x_proj_w[:, :DT_RANK, :])
    lhsT_M2 = np.zeros((D, K, C_IN), bf16)
    for k in range(K):
        lhsT_M2[:, k, 0:D] = M[k].T
        lhsT_M2[:, k, D:C_IN] = M[k].T
    dtb2 = np.zeros((C_IN, K), np.float32)
    dtb2[0:D] = dt_proj_b.T
    dtb2[D:C_IN] = dt_proj_b.T

    lhsT_st2 = np.zeros((D, 33), bf16)
    lhsT_st2[0:C_H, 0] = bf16(1.0 / C_H)
    lhsT_st2[C_H:D, 32] = bf16(1.0 / C_H)
    lhsT_bc128 = np.ones((1, C_IN), bf16)
    lhsT_rm2 = np.zeros((33, C_IN), bf16)
    lhsT_rm2[0, 0:D] = 1.0
    lhsT_rm2[32, D:C_IN] = 1.0
    lhsT_bcg = final_ln_w.reshape(1, C_IN).astype(bf16)
    lhsT_ys = np.zeros((C_IN, D), bf16)
    for p in range(C_IN):
        lhsT_ys[p, p % D] = 1.0
    lhsT_st64 = np.zeros((C_IN, 33), bf16)
    lhsT_st64[0:D, 0] = bf16(1.0 / D)
    lhsT_st64[D:C_IN, 32] = bf16(1.0 / D)
    lhsT_s128 = np.full((C_IN, 1), 1.0 / C_IN, bf16)

    common = {
        "lhsT_ip2": lhsT_ip2, "lhsT_nw2": lhsT_nw2,
        "bias_xx": bias_ip[0:D].reshape(D, 1),
        "bias_z": bias_ip[D:C_IN].reshape(D, 1),
        "w9d": w9d, "convb": conv_b.reshape(D, 1),
        "lhsT_st2": lhsT_st2, "lhsT_bc128": lhsT_bc128,
        "lhsT_rm2": lhsT_rm2, "lhsT_bcg": lhsT_bcg,
        "lhsT_M2": lhsT_M2, "dtb2": dtb2,
        "lhsT_ys": lhsT_ys, "lhsT_Dsq": lhsT_Dsq,
        "lhsT_st64": lhsT_st64, "lhsT_op2": lhsT_op2,
        "lhsT_s128": lhsT_s128,
        "gamma": final_ln_w.reshape(C_IN, 1),
        "beta": final_ln_b.reshape(C_IN, 1),
    }

    g = HEAD
    cg = C_IN // HEAD
    per_b = []
    for b in range(B):
        xs = x[b].reshape(H, W, g, cg).transpose(0, 1, 3, 2).reshape(L, C_IN)
        per_b.append(np.ascontiguousarray(xs.T).astype(bf16))  # (128, L)

    in_maps = []
    for c in range(NCORES):
        b, nh = c // 4, c % 4
        A2 = np.zeros((C_IN, K, 2), np.float32)
        lhsT_B = np.zeros((D, K, 2, C_IN), bf16)
        lhsT_C = np.zeros((D, K, 2, C_IN), bf16)
        for k in range(K):
            for gl in range(2):
                for half in range(2):
                    n = 4 * nh + 2 * gl + half
                    rows = slice(64 * half, 64 * half + 64)
                    A2[rows, k, gl] = A[k, :, n]
                    lhsT_B[:, k, gl, rows] = \
                        x_proj_w[k, DT_RANK + n, :][:, None].astype(bf16)
                    lhsT_C[:, k, gl, rows] = \
                        x_proj_w[k, DT_RANK + N + n, :][:, None].astype(bf16)
        in_maps.append(dict(common, x_shuf=per_b[b],
                            A2=A2, lhsT_B=lhsT_B, lhsT_C=lhsT_C))
    vs = float(np.asarray(inputs["vss_skip"]).ravel()[0])
    cvm = float(np.asarray(inputs["cvm_skip"]).ravel()[0])
    return in_maps, vs, cvm


def kernel(**inputs) -> np.ndarray:
    from concourse.bass_utils import run_bass_kernel_spmd

    in_maps, vs, cvm = _host_prep(inputs)
    key = (vs, cvm)
    if key not in _cache:
        _cache[key] = _build(vs, cvm)
    nc = _cache[key]
    res = run_bass_kernel_spmd(nc, in_maps, core_ids=list(range(NCORES)))
    out = np.zeros((B, H, W, C_IN), np.float32)
    for b in range(B):
        out_cf = res.results[4 * b]["out_cf"]  # (128, L)
        out[b] = out_cf.T.reshape(H, W, C_IN)
    return out
